# revision 11
# baseline (speedup 1.0000x reference)
"""CAML (conv attention for multi-label) Trainium2 Bass kernel.

Model: y[b,l] = sum_h out_w[l,h] * m[b,l,h] + out_b[l]
  where m = softmax_s(u_w @ h^T) @ h,  h = tanh(conv1d(emb[x])).

Strategy (8 cores, tensor-parallel over labels):
  - labels padded 8921 -> 8928, each core takes a 1116-label shard of
    u_w / out_w / out_b (pre-transposed on host); x/emb/conv replicated.
  - per core, per batch:
    * embedding gather: indirect DMA, 128 tokens/call, rows padded to
      512B on host for line-rate DMA.
    * PE transpose e-chunks -> e^T (E on partitions), conv1d = 9
      accumulating matmuls per 512-col PSUM chunk, tanh (+bias) on ACT.
    * attention without softmax-max (scores are tiny: |s| < 0.01 for
      this model scale, exp is safe) and without materializing alpha:
        scores^T chunk (128 s, L) = h^T_chunk.T @ u_w^T      (PE)
        E = exp(scores^T)                                    (ACT)
        m_aug^T (51, L) += [h|1]_chunk.T @ E                 (PE, accum)
      row 50 of m_aug^T is the softmax denominator (ones-augmented),
      division is applied after the contraction (linearity).
    * y = (1_50.T @ (m^T * out_w^T)) / denom + out_b  -> DMA out.
  - matmuls run in float32r (1 cycle/row at N>=256 vs 4 for fp32).

kernel(**inputs) takes the full unsharded inputs and returns the full
(8, 8921) output; sharding + gather happen inside.
"""

import numpy as np

P = 128


def default_cfg():
    return dict(
        B=8,
        S=2000,
        SP=2048,       # padded seq (multiple of 512)
        E=100,
        EP=128,        # padded emb row (512B)
        H=50,
        K=9,
        LC=1116,       # labels per core
        NSW=372,       # matmul N-slice width (LC % NSW == 0, NSW <= 512)
        CW=512,        # conv psum chunk width
        mm_dtype="float32r",
    )


def build_nc(cfg, num_devices=8):
    import concourse.bacc as bacc
    import concourse.bass as bass
    import concourse.mybir as mybir
    import concourse.tile as tile
    from concourse.masks import make_identity

    B, S, SP, E, EP, H, K, LC, NSW, CW = (
        cfg["B"], cfg["S"], cfg["SP"], cfg["E"], cfg["EP"], cfg["H"],
        cfg["K"], cfg["LC"], cfg["NSW"], cfg["CW"],
    )
    V = cfg["V"]
    f32 = mybir.dt.float32
    mmdt = getattr(mybir.dt, cfg["mm_dtype"])
    NCH = SP // P                  # s-chunks per batch
    NCC = SP // CW                 # conv chunks per batch
    TPC = CW // P                  # transposes per conv chunk group
    NSL = LC // NSW                # N slices
    G = B * NCH                    # total gather chunks
    HB1 = H + 1
    ONE = 0                        # ones-column index -> denom lands on partition 0

    nc = bacc.Bacc("TRN2", target_bir_lowering=False, debug=False,
                   num_devices=num_devices)

    xw_d = nc.dram_tensor("xw", [P, G], mybir.dt.int32, kind="ExternalInput")
    emb_d = nc.dram_tensor("emb", [V, EP], f32, kind="ExternalInput")
    w_d = nc.dram_tensor("wT", [E, K * H], f32, kind="ExternalInput")
    cb_d = nc.dram_tensor("convb", [H, 1], f32, kind="ExternalInput")
    u_d = nc.dram_tensor("uT", [H, LC], f32, kind="ExternalInput")
    ow_d = nc.dram_tensor("owT", [HB1, LC], f32, kind="ExternalInput")
    ob_d = nc.dram_tensor("ob", [1, LC], f32, kind="ExternalInput")
    y_d = nc.dram_tensor("y", [B, LC], f32, kind="ExternalOutput")
    dbg = cfg.get("debug_dump")
    if dbg:
        eT_d = nc.dram_tensor("dbg_eT", [E, SP + 8], f32, kind="ExternalOutput")
        hT_d = nc.dram_tensor("dbg_hT", [H, SP], f32, kind="ExternalOutput")
        hb_d = nc.dram_tensor("dbg_hb", [P, NCH * 64], f32, kind="ExternalOutput")
        m_d = nc.dram_tensor("dbg_m", [HB1, LC], f32, kind="ExternalOutput")
        ec_d = nc.dram_tensor("dbg_ec", [P, EP], f32, kind="ExternalOutput")


    with tile.TileContext(nc) as tc:
        with (
            tc.tile_pool(name="const", bufs=1) as const,
            tc.tile_pool(name="gat", bufs=6) as gat,
            tc.tile_pool(name="eT", bufs=2) as eT_p,
            tc.tile_pool(name="hT", bufs=3) as hT_p,
            tc.tile_pool(name="hb", bufs=2) as hb_p,
            tc.tile_pool(name="Ep", bufs=3) as E_p,
            tc.tile_pool(name="tail", bufs=2) as tail_p,
            tc.tile_pool(name="pmisc", bufs=2, space="PSUM") as pmisc,
            tc.tile_pool(name="psc", bufs=1, space="PSUM") as psc,
            tc.tile_pool(name="pm", bufs=1, space="PSUM") as pm,
        ):
            ident = const.tile([P, P], f32)
            make_identity(nc, ident[:])
            os_f = const.tile([HB1, 1], f32)
            nc.vector.memset(os_f[:], 1.0)
            nc.vector.memset(os_f[0:1, :], 0.0)
            ones_sel = const.tile([HB1, 1], mmdt)
            nc.vector.tensor_copy(out=ones_sel[:], in_=os_f[:])
            ez_f = const.tile([E, 4], f32)
            nc.vector.memset(ez_f[:], 0.0)
            hbone_f = const.tile([P, NCH, 1], f32)
            nc.vector.memset(hbone_f[:], 1.0)
            hblast_f = const.tile([P, HB1], f32)
            nc.vector.memset(hblast_f[:], 0.0)
            if S % P:
                nc.vector.memset(hblast_f[:S % P, 0:1], 1.0)

            xw = const.tile([P, G], mybir.dt.int32)
            nc.sync.dma_start(out=xw[:], in_=xw_d[:])
            w_f = const.tile([E, K * H], f32)
            nc.sync.dma_start(out=w_f[:], in_=w_d[:])
            w_sb = const.tile([E, K * H], mmdt)
            nc.vector.tensor_copy(out=w_sb[:], in_=w_f[:])
            cb_sb = const.tile([H, 1], f32)
            nc.sync.dma_start(out=cb_sb[:], in_=cb_d[:])
            u_f = const.tile([H, LC], f32)
            nc.sync.dma_start(out=u_f[:], in_=u_d[:])
            u_sb = const.tile([H, LC], mmdt)
            nc.vector.tensor_copy(out=u_sb[:], in_=u_f[:])
            ow_sb = const.tile([HB1, LC], f32)
            nc.sync.dma_start(out=ow_sb[:], in_=ow_d[:])
            ob_sb = const.tile([1, LC], f32)
            nc.sync.dma_start(out=ob_sb[:], in_=ob_d[:])

            for b in range(B):
                # ---- embedding gather + transpose -> eT (E, SP+8) ----
                eT = eT_p.tile([E, SP + 8], mmdt, tag="eT")
                nc.vector.tensor_copy(out=eT[:, 0:4], in_=ez_f[:])
                nc.vector.tensor_copy(out=eT[:, SP + 4:SP + 8], in_=ez_f[:])
                for g in range(NCH // TPC):
                    ps_tr = pmisc.tile([E, CW], f32, tag="pmisc")
                    for j in range(TPC):
                        c = g * TPC + j
                        ec = gat.tile([P, EP], f32, tag="gat")
                        nc.gpsimd.indirect_dma_start(
                            out=ec[:],
                            out_offset=None,
                            in_=emb_d[:],
                            in_offset=bass.IndirectOffsetOnAxis(
                                ap=xw[:, b * NCH + c: b * NCH + c + 1], axis=0
                            ),
                        )
                        if dbg and b == 0 and c == 0:
                            nc.sync.dma_start(out=ec_d[:], in_=ec[:])
                        nc.tensor.transpose(
                            out=ps_tr[:, j * P:(j + 1) * P],
                            in_=ec[:, :E],
                            identity=ident[:],
                        )
                    nc.vector.tensor_copy(
                        out=eT[:, 4 + g * CW: 4 + (g + 1) * CW], in_=ps_tr[:]
                    )

                if dbg and b == 0:
                    nc.sync.dma_start(out=eT_d[:], in_=eT[:].bitcast(f32))
                # ---- conv1d + tanh -> hT (H, SP) ----
                hT = hT_p.tile([H, SP], mmdt, tag="hT")
                for cc in range(NCC):
                    ps_cv = pmisc.tile([H, CW], f32, tag="pmisc")
                    for k in range(K):
                        nc.tensor.matmul(
                            out=ps_cv[:],
                            lhsT=w_sb[:, k * H:(k + 1) * H],
                            rhs=eT[:, k + cc * CW: k + cc * CW + CW],
                            start=(k == 0),
                            stop=(k == K - 1),
                        )
                    nc.scalar.activation(
                        out=hT[:, cc * CW:(cc + 1) * CW],
                        in_=ps_cv[:],
                        func=mybir.ActivationFunctionType.Tanh,
                        bias=cb_sb[:],
                        scale=1.0,
                    )

                # ---- hb (P, NCH, 64): [h | 1 | pad], masked past S ----
                # SBUF APs must start at partition 0/32/64/96, so mask the
                # padded tail of the last chunk by pre-zeroing it and writing
                # only the valid row range (all writes partition-0 based).
                SREM = S % P
                LAST = NCH - 1
                hb = hb_p.tile([P, NCH, 64], mmdt, tag="hb")
                nc.vector.tensor_copy(out=hb[:, :, 0:1], in_=hbone_f[:])
                if SREM:
                    nc.vector.tensor_copy(out=hb[:, LAST, :HB1], in_=hblast_f[:])
                for g in range(NCH // TPC):
                    ps_hb = pmisc.tile([P, TPC, H], f32, tag="pmisc")
                    for j in range(TPC):
                        c = g * TPC + j
                        nc.tensor.transpose(
                            out=ps_hb[:, j, :],
                            in_=hT[:, c * P:(c + 1) * P].bitcast(f32),
                            identity=ident[:H, :H],
                        )
                    gs = g * TPC
                    # ones col is at 0; h occupies cols 1..H of each chunk
                    if SREM and gs + TPC - 1 == LAST:
                        if TPC > 1:
                            nc.vector.tensor_copy(
                                out=hb[:, gs:LAST, 1:HB1], in_=ps_hb[:, :TPC - 1, :]
                            )
                        nc.vector.tensor_copy(
                            out=hb[:SREM, LAST, 1:HB1], in_=ps_hb[:SREM, TPC - 1, :]
                        )
                    else:
                        nc.vector.tensor_copy(
                            out=hb[:, gs:gs + TPC, 1:HB1], in_=ps_hb[:, :TPC]
                        )

                if dbg and b == 0:
                    nc.sync.dma_start(out=hT_d[:], in_=hT[:].bitcast(f32))
                    nc.sync.dma_start(out=hb_d[:], in_=hb[:].bitcast(f32).rearrange("p a b -> p (a b)"))
                # ---- attention: scores^T -> exp -> m_aug accumulate ----
                # one PSUM tile (= one accumulation group / bank) per N-slice
                ps_ms = [pm.tile([HB1, NSW], f32, tag=f"pm{j}", name=f"ps_m{j}") for j in range(NSL)]
                for c in range(NCH):
                    # one PSUM bank (512 f32) per N-slice: a matmul output
                    # must never cross a bank boundary (silent corruption)
                    ps_sc = psc.tile([P, NSL, 512], f32, tag="psc")
                    for j in range(NSL):
                        nc.tensor.matmul(
                            out=ps_sc[:, j, :NSW],
                            lhsT=hT[:, c * P:(c + 1) * P],
                            rhs=u_sb[:, j * NSW:(j + 1) * NSW],
                            start=True,
                            stop=True,
                        )
                    Et = E_p.tile([P, NSL, NSW], mmdt, tag="Ep")
                    nc.scalar.activation(
                        out=Et[:], in_=ps_sc[:, :, :NSW],
                        func=mybir.ActivationFunctionType.Exp,
                    )
                    for j in range(NSL):
                        nc.tensor.matmul(
                            out=ps_ms[j][:],
                            lhsT=hb[:, c, :HB1],
                            rhs=Et[:, j, :],
                            start=(c == 0),
                            stop=(c == NCH - 1),
                        )

                # ---- tail: y = (1.T @ (m^T * owT)) / denom + ob ----
                recip = tail_p.tile([1, LC], f32, tag="recip")
                prod = tail_p.tile([HB1, LC], mmdt, tag="prod")
                for j in range(NSL):
                    sl = slice(j * NSW, (j + 1) * NSW)
                    nc.vector.tensor_mul(out=prod[:, sl], in0=ps_ms[j][:HB1, :],
                                         in1=ow_sb[:, sl])
                    nc.vector.reciprocal(out=recip[:, sl],
                                         in_=prod[ONE:ONE + 1, sl])
                ps_ys = [pm.tile([1, NSW], f32, tag=f"pm{j}", name=f"ps_y{j}") for j in range(NSL)]
                for j in range(NSL):
                    nc.tensor.matmul(
                        out=ps_ys[j][:],
                        lhsT=ones_sel[:],
                        rhs=prod[:, j * NSW:(j + 1) * NSW],
                        start=True,
                        stop=True,
                    )
                if dbg and b == 0:
                    nc.sync.dma_start(out=m_d[:], in_=prod[:].bitcast(f32))
                y_sb = tail_p.tile([1, LC], f32, tag="ysb")
                for j in range(NSL):
                    sl = slice(j * NSW, (j + 1) * NSW)
                    nc.vector.tensor_mul(out=y_sb[:, sl], in0=ps_ys[j][:],
                                         in1=recip[:, sl])
                nc.vector.tensor_add(out=y_sb[:], in0=y_sb[:], in1=ob_sb[:])
                nc.sync.dma_start(out=y_d[b:b + 1, :], in_=y_sb[:])

    nc.compile()
    return nc


def prep_inputs(cfg, x, emb_table, conv_w, conv_b, u_w, out_w, out_b):
    """Host-side (data-independent) reshapes/padding + label sharding."""
    B, S, SP, E, EP, H, K, LC = (
        cfg["B"], cfg["S"], cfg["SP"], cfg["E"], cfg["EP"], cfg["H"],
        cfg["K"], cfg["LC"],
    )
    ncores = cfg["ncores"]
    V = emb_table.shape[0]
    L = u_w.shape[0]
    LPAD = LC * ncores

    x = np.asarray(x).astype(np.int32)
    x_pad = np.zeros((B, SP), np.int32)
    x_pad[:, :S] = x
    xw = np.ascontiguousarray(x_pad.reshape(-1, P).T)  # (128, G)

    emb_pad = np.zeros((V, EP), np.float32)
    emb_pad[:, :E] = emb_table
    w_host = np.ascontiguousarray(
        np.asarray(conv_w, np.float32).transpose(1, 2, 0).reshape(E, K * H)
    )
    cb = np.ascontiguousarray(np.asarray(conv_b, np.float32).reshape(H, 1))

    u_pad = np.zeros((LPAD, H), np.float32)
    u_pad[:L] = u_w
    ow_pad = np.zeros((LPAD, H), np.float32)
    ow_pad[:L] = out_w
    ob_pad = np.zeros(LPAD, np.float32)
    ob_pad[:L] = out_b

    ONE = 32
    in_maps = []
    for c in range(ncores):
        sl = slice(c * LC, (c + 1) * LC)
        owT = ow_pad[sl].T          # (H, LC)
        ow2 = np.ones((H + 1, LC), np.float32)
        ow2[1:H + 1] = owT
        in_maps.append({
            "xw": xw,
            "emb": emb_pad,
            "wT": w_host,
            "convb": cb,
            "uT": np.ascontiguousarray(u_pad[sl].T),
            "owT": np.ascontiguousarray(ow2),
            "ob": np.ascontiguousarray(ob_pad[sl].reshape(1, LC)),
        })
    return in_maps


_NC_CACHE = {}


def kernel(x, emb_table, conv_w, conv_b, u_w, out_w, out_b):
    from concourse import bass_utils

    cfg = default_cfg()
    cfg["ncores"] = 8
    cfg["V"] = int(np.asarray(emb_table).shape[0])
    L = int(np.asarray(u_w).shape[0])

    key = (cfg["V"], L)
    if key not in _NC_CACHE:
        _NC_CACHE[key] = build_nc(cfg, num_devices=cfg["ncores"])
    nc = _NC_CACHE[key]

    in_maps = prep_inputs(cfg, x, emb_table, conv_w, conv_b, u_w, out_w, out_b)
    res = bass_utils.run_bass_kernel_spmd(
        nc, in_maps, core_ids=list(range(cfg["ncores"]))
    )
    y = np.concatenate([res.results[c]["y"] for c in range(cfg["ncores"])], axis=1)
    return np.ascontiguousarray(y[:, :L])
